# revision 31
# baseline (speedup 1.0000x reference)
"""CorefScore kernel for 8 Trainium2 NeuronCores.

Shards the mention axis M=2048 across 8 cores (256 owned mentions plus a
64-row halo of preceding mentions). The banded pairwise MLP runs in
"superrounds" of up to 4 rounds (2 deltas each): the DVE emits one batched
shifted-product tensor_tensor per d-tile per superround ([128, 8*256] fp16,
2x perf mode); the PE contracts them with W1c in fp16 (fp32 PSUM); Ya+shifted
Yb sums are merged on the DVE (c1/c2) and injected via identity matmuls. The
22-wide h2 half of all rounds packs into one PSUM bank at partition offsets
0/32/64/96 via col-tiled matmuls on disjoint PE column strips. Pair scores
(w2p contraction) use diagonal 32-strips, packed per superround, respread via
DMA + PE transposes. v1: preamble packed into 5 weight streams (40 matmuls),
batched input DMAs with fine-grained xt chunks, first a1 round emitted ahead
of the deferred pair matmuls, product tile t=7 emitted on GpSimd to unload
the DVE, single fused output DMA.
"""

import os
import sys

import numpy as np

for _p in ("/opt/trn_rl_repo", "/opt/pypackages"):
    if os.path.isdir(_p) and _p not in sys.path:
        sys.path.append(_p)

import concourse.bacc as bacc
import concourse.bass as bass
import concourse.mybir as mybir
import concourse.tile as tile
from concourse.ap import AP
from concourse.bass_utils import run_bass_kernel_spmd

F16 = mybir.dt.float16
F32 = mybir.dt.float32
AF = mybir.ActivationFunctionType

M, D, H, K = 2048, 900, 150, 50
NCORES = 8
MC = M // NCORES          # owned mentions per core
HB = 64                   # halo columns (>= K)
W = MC + HB               # X^T window width per core
DP = 1024                 # padded feature dim (8 tiles of 128)
NDT = DP // 128           # number of d tiles
H1, H2 = 128, H - 128     # h split
# superround sizes (rounds of 2 deltas each); sum = 25 rounds = 50 deltas;
# first kept small so the serial DVE product chain gating pipeline fill is
# short; last small-ish so the epilogue pair/respread chain is short
SRS = [2, 4, 4, 4, 4, 4, 3]
GPS_T7 = False            # GpSimd tensor_tensor measured 2-8x slower + DVE
                          # SBUF-port contention; keep everything on the DVE

_cache = {}


def _ap3(t_ap, p_lo, p_n, off, dims, pstep=1):
    """3-D free-dim view of a tile AP: partitions [p_lo, p_lo+p_n*pstep) with
    partition step pstep, free offset `off` elements, free dims."""
    b = t_ap[p_lo:p_lo + 1, 0:1]
    pstride = b.ap[0][0]
    return AP(b.tensor, b.offset + off,
              [[pstride * pstep, p_n]] + [list(d) for d in dims])


def _build():
    nc = bacc.Bacc("TRN2", target_bir_lowering=False, debug=False)

    xt_d = nc.dram_tensor("xt", [DP, W], F16, kind="ExternalInput").ap()
    wh1_d = nc.dram_tensor("wh1", [DP, 384], F16, kind="ExternalInput").ap()
    wh2_d = nc.dram_tensor("wh2", [DP, 76], F16, kind="ExternalInput").ap()
    w1c_d = nc.dram_tensor("w1c", [DP, H], F16, kind="ExternalInput").ap()
    bias_d = nc.dram_tensor("bias_all", [128, 4], F32, kind="ExternalInput").ap()
    sm16_d = nc.dram_tensor("sm16", [128, 132], F16, kind="ExternalInput").ap()
    mm_d = nc.dram_tensor("mm_ma", [128, 2 * (K + 1)], F32,
                          kind="ExternalInput").ap()
    out_d = nc.dram_tensor("out", [MC, K + 1], F32, kind="ExternalOutput").ap()

    def dma3(eng, dst_tile, src_dram, cols, t0=0, t1=NDT):
        """One DMA loading d-tiles [t0, t1) of [DP, cols] dram into a
        [128, NDT*cols] tile at the matching free offset."""
        src = AP(src_dram.tensor, src_dram.offset + 128 * cols * t0,
                 [[cols, 128], [128 * cols, t1 - t0], [1, cols]])
        eng.dma_start(out=dst_tile[:, cols * t0:cols * t1], in_=src)

    with tile.TileContext(nc) as tc:
        with (
            tc.tile_pool(name="cp", bufs=1) as cp,
            tc.tile_pool(name="wp", bufs=2) as wp,
            tc.tile_pool(name="pp", bufs=2, space="PSUM") as pp,
        ):
            # ---- input loads ----
            # Two HWDGE queues, ordered so the two lead-in critical paths
            # (DVE product chain over xt, and wh1 -> psya/psyb -> c1) finish
            # together: sync carries xt tiles 0-5 + sm16 + w1c, scalar carries
            # wh1 + bias + xt tiles 6-7 + the rest.
            xt = cp.tile([128, NDT * W], F16, tag="xt")
            dma3(nc.sync, xt, xt_d, W, 0, 2)
            dma3(nc.sync, xt, xt_d, W, 2, 4)
            dma3(nc.sync, xt, xt_d, W, 4, 6)
            sm16 = cp.tile([128, 132], F16, tag="sm16")
            nc.sync.dma_start(out=sm16[:], in_=sm16_d[:])
            w1c = cp.tile([128, NDT * H], F16, tag="w1c")
            dma3(nc.sync, w1c, w1c_d, H, 0, 4)
            dma3(nc.sync, w1c, w1c_d, H, 4, NDT)
            bias = cp.tile([128, 4], F32, tag="bias")
            nc.scalar.dma_start(out=bias[:], in_=bias_d[:])
            wh1 = cp.tile([128, NDT * 384], F16, tag="wh1")
            dma3(nc.scalar, wh1, wh1_d, 384, 0, 2)
            dma3(nc.scalar, wh1, wh1_d, 384, 2, 5)
            dma3(nc.scalar, wh1, wh1_d, 384, 5, NDT)
            dma3(nc.scalar, xt, xt_d, W, 6, NDT)
            wh2 = cp.tile([128, NDT * 76], F16, tag="wh2")
            dma3(nc.scalar, wh2, wh2_d, 76)
            mm_ma = cp.tile([128, 2 * (K + 1)], F32, tag="mm_ma")
            nc.scalar.dma_start(out=mm_ma[:], in_=mm_d[:])

            w2 = sm16[:, 0:4]
            idn = sm16[:, 4:132]

            def wsl(wt, stride, t, ho, hn):  # stationary slice of a weight tile
                return wt[:, stride * t + ho:stride * t + ho + hn]

            # ---- preamble MLPs: 5 packed streams, split in two parts ----
            # part 1 (now): psya/psyb -> ya1/yb1 -> c1(0). The other three
            # streams (ment + h2 halves) defer into SR0 where they fill the
            # PE bubbles left while a1 chases the DVE product chain.
            # wh1 = [w1a_h1 | w1b_h1 | w1m_h1], wh2 = [ya2 | pad | ma2 | yb2]
            psya1 = pp.tile([H1, MC], F32, tag="ah1")
            psyb1 = pp.tile([H1, W], F32, tag="ah1")
            for t in range(NDT):
                nc.tensor.matmul(psya1[:], wsl(wh1, 384, t, 0, 128),
                                 xt[:, W * t + HB:W * (t + 1)],
                                 start=(t == 0), stop=(t == NDT - 1))
            for t in range(NDT):
                nc.tensor.matmul(psyb1[:], wsl(wh1, 384, t, 128, 128),
                                 xt[:, W * t:W * (t + 1)],
                                 start=(t == 0), stop=(t == NDT - 1))
            ya1 = cp.tile([H1, MC], F16, tag="ya1")
            nc.scalar.activation(ya1[:], psya1[:], AF.Identity, bias=bias[:, 2:3])
            yb1 = cp.tile([H1, W], F16, tag="yb1")
            nc.scalar.copy(yb1[:], psyb1[:])

            # tiles written by the deferred preamble part (allocated up front
            # so earlier-emitted code can reference them; instruction order
            # still follows emission order)
            ya2 = cp.tile([H2, MC], F16, tag="ya2")
            ma1 = cp.tile([H1, W], F16, tag="ma1")
            ma2t = cp.tile([54, W], F16, tag="ma2t")
            yb2 = cp.tile([H2, W], F16, tag="yb2")
            ment_row = cp.tile([32, W], F16, tag="ment_row")
            nc.vector.memset(ment_row[:], 0.0)
            mcol0 = cp.tile([128, 1], F32, tag="mcol0")
            mcol1 = cp.tile([128, 1], F32, tag="mcol1")
            ment_col = [mcol0, mcol1]
            scm = cp.tile([128, 2 * (K + 1)], F32, tag="scm")
            nc.vector.memset(scm[:], 0.0)

            def emit_preamble_tail():
                psm1 = pp.tile([H1, W], F32, tag="ah2")
                psX = pp.tile([54, W], F32, tag="ah2")
                psyb2 = pp.tile([H2, W], F32, tag="psp")
                for t in range(NDT):
                    nc.tensor.matmul(psX[:], wsl(wh2, 76, t, 0, 54),
                                     xt[:, W * t:W * (t + 1)],
                                     start=(t == 0), stop=(t == NDT - 1))
                for t in range(NDT):
                    nc.tensor.matmul(psyb2[:], wsl(wh2, 76, t, 54, 22),
                                     xt[:, W * t:W * (t + 1)],
                                     start=(t == 0), stop=(t == NDT - 1))
                for t in range(NDT):
                    nc.tensor.matmul(psm1[:], wsl(wh1, 384, t, 256, 128),
                                     xt[:, W * t:W * (t + 1)],
                                     start=(t == 0), stop=(t == NDT - 1))
                nc.scalar.activation(ya2[:], psX[0:H2, HB:], AF.Identity,
                                     bias=bias[0:H2, 3:4])
                nc.scalar.copy(yb2[:], psyb2[:])
                nc.scalar.activation(ma1[:], psm1[:], AF.Relu, bias=bias[:, 0:1])
                nc.scalar.activation(ma2t[32:54, :], psX[32:54, :], AF.Relu,
                                     bias=bias[32:54, 1:2])
                # mention score row over the window
                psme = pp.tile([1, W], F32, tag="psp")
                nc.tensor.matmul(psme[:], w2[:, 0:1], ma1[:], start=True,
                                 stop=False)
                nc.tensor.matmul(psme[:], w2[32:54, 1:2], ma2t[32:54, :],
                                 start=False, stop=True, tile_position=(32, 0))
                # row 0 = mention scores; rows 1..31 stay zero so the
                # e0-column identity-inject matmul picks row 0 only
                nc.scalar.copy(ment_row[0:1, :], psme[:])
                # ment as per-partition columns for the owned mention blocks
                for mb in range(2):
                    pst = pp.tile([128, 1], F16, tag="pre")
                    nc.tensor.transpose(
                        pst[:],
                        ment_row[0:1, HB + 128 * mb:HB + 128 * (mb + 1)],
                        idn[0:1, 0:1])
                    nc.scalar.copy(ment_col[mb][:], pst[:])

            # ---- DVE: batched shifted products per superround ----
            # SR s covers rounds r0..r0+R-1; delta of column group j (0..2R-1)
            # is d0 - j with d0 = K - 2*r0; product col j*MC+m multiplies
            # X^T[., m] (owned) by X^T[., m - (d0 - j)].
            r0s = []
            acc = 0
            for R in SRS:
                r0s.append(acc)
                acc += R

            pts_q = {}

            def emit_products(ss):
                """One batched product op per d-tile covering the superrounds
                in `ss` (their delta ranges are contiguous)."""
                d0 = K - 2 * r0s[ss[0]]
                RT = sum(SRS[s] for s in ss)
                pts = []
                for t in range(NDT):
                    pt = wp.tile([128, 2 * RT * MC], F16, tag=f"pt{t}",
                                 padded_shape=[128, 8 * MC])
                    nc.vector.tensor_tensor(
                        _ap3(pt[:], 0, 128, 0, [(MC, 2 * RT), (1, MC)]),
                        _ap3(xt[:], 0, 128, W * t + HB, [(0, 2 * RT), (1, MC)]),
                        _ap3(xt[:], 0, 128, W * t + HB - d0,
                             [(1, 2 * RT), (1, MC)]),
                        mybir.AluOpType.mult)
                    pts.append(pt)
                off = 0
                for s in ss:
                    pts_q[s] = (pts, off)
                    off += 2 * SRS[s] * MC

            def emit_c2(s):
                R = SRS[s]
                d0 = K - 2 * r0s[s]
                c2 = wp.tile([H2, 2 * R * MC], F16, tag="c2",
                             padded_shape=[128, 8 * MC])
                nc.vector.tensor_tensor(
                    _ap3(c2[:], 0, H2, 0, [(MC, 2 * R), (1, MC)]),
                    _ap3(ya2[:], 0, H2, 0, [(0, 2 * R), (1, MC)]),
                    _ap3(yb2[:], 0, H2, HB - d0, [(1, 2 * R), (1, MC)]),
                    mybir.AluOpType.add)
                return c2

            def emit_c1(s):
                R = SRS[s]
                d0 = K - 2 * r0s[s]
                c1 = wp.tile([H1, 2 * R * MC], F16, tag="c1",
                             padded_shape=[128, 8 * MC])
                nc.vector.tensor_tensor(
                    _ap3(c1[:], 0, H1, 0, [(MC, 2 * R), (1, MC)]),
                    _ap3(ya1[:], 0, H1, 0, [(0, 2 * R), (1, MC)]),
                    _ap3(yb1[:], 0, H1, HB - d0, [(1, 2 * R), (1, MC)]),
                    mybir.AluOpType.add)
                return c1

            # per-SR product emission: merging SR pairs into bigger ops makes
            # the DVE FIFO lumpy and head-of-line-blocks the next SR's c1/c2.
            # c2(0) is emitted inside SR0 after the deferred preamble writes
            # ya2/yb2, placed behind products(1) in the DVE FIFO.
            PGROUPS = {s: (s,) for s in range(len(SRS))}
            emit_products(PGROUPS[0])
            c1_q = {0: emit_c1(0)}
            c2_q = {}
            emit_products(PGROUPS[1])

            # ---- superround loop state ----
            prev = None       # state of SR s-1 for deferred pair work

            def emit_pairs_for(state, respread_dma=True):
                """PE pair matmuls + evac + respread DMA for SR s-1."""
                s, R, a1s, a2xs = state
                d0 = K - 2 * r0s[s]
                psp = pp.tile([97, MC * 2], F32, tag="psp")
                # stage-ordered across the 4 column strips so the strips run
                # concurrently: all ment_j opens, then all w2-a1, then w2-a2x.
                for g in range(R):
                    # ment_j first (start=True): a strided-moving matmul with
                    # start=False wedges the exec unit, so it opens the group
                    nc.tensor.matmul(
                        psp[32 * g:32 * g + 1, :], idn[0:32, 0:1],
                        _ap3(ment_row[:], 0, 32, HB - (d0 - 2 * g),
                             [(1, 2), (1, MC)]),
                        start=True, stop=False, tile_position=(0, 32 * g))
                for g in range(R):
                    nc.tensor.matmul(psp[32 * g:32 * g + 1, :], w2[:, 2:3],
                                     a1s[g][:], start=False, stop=False,
                                     tile_position=(0, 32 * g))
                for g in range(R):
                    nc.tensor.matmul(psp[32 * g:32 * g + 1, :],
                                     w2[32 * g:32 * g + H2, 3:4],
                                     a2xs[32 * g:32 * g + H2, :],
                                     start=False, stop=True,
                                     tile_position=(32 * g, 32 * g))
                pair_sb = wp.tile([97, MC * 2], F16, tag="pair_sb")
                nc.scalar.copy(pair_sb[:], psp[:])
                if not respread_dma:
                    return pair_sb
                pairK = wp.tile([2 * R, MC], F16, tag="pairK",
                                padded_shape=[128, MC])
                nc.sync.dma_start(
                    out=pairK[:],
                    in_=_ap3(pair_sb[:], 0, R, 0, [(MC, 2), (1, MC)], pstep=32))
                return pairK

            def emit_respread_tail(state, pairK):
                """Transposes + scM chunk evac for SR s-1 (after its DMA)."""
                s, R, a1s, a2xs = state
                k0 = 2 * r0s[s]
                cn = 2 * R
                for mb in range(2):
                    ptr = pp.tile([128, cn], F16, tag="pre",
                                  padded_shape=[128, 8])
                    nc.tensor.transpose(ptr[:], pairK[0:cn, 128 * mb:128 * (mb + 1)],
                                        idn[0:cn, 0:cn])
                    nc.scalar.activation(
                        scm[:, (K + 1) * mb + k0:(K + 1) * mb + k0 + cn],
                        ptr[:], AF.Identity, bias=ment_col[mb][:])

            def emit_a1_round(g, pts, poff, c1):
                ps1 = pp.tile([H1, 2 * MC], F32, tag="ah1")
                for t in range(NDT):
                    nc.tensor.matmul(ps1[:], wsl(w1c, H, t, 0, H1),
                                     pts[t][:, poff + 2 * g * MC:
                                            poff + (2 * g + 2) * MC],
                                     start=(t == 0), stop=False)
                nc.tensor.matmul(
                    ps1[:], idn[0:128, 0:128],
                    c1[:, 2 * g * MC:(2 * g + 2) * MC],
                    start=False, stop=True)
                a1 = wp.tile([H1, 2 * MC], F16, tag="a1", bufs=8)
                nc.scalar.activation(a1[:], ps1[:], AF.Relu)
                return a1

            for s, R in enumerate(SRS):
                pts, poff = pts_q.pop(s)
                c1 = c1_q.pop(s)

                # first round ahead of the deferred pair matmuls so the PE has
                # work while the previous SR's a2x evac lands
                a1s = [emit_a1_round(0, pts, poff, c1)]
                pairK_prev = emit_pairs_for(prev) if prev is not None else None
                for g in range(1, R):
                    a1s.append(emit_a1_round(g, pts, poff, c1))

                if s == 0:
                    emit_preamble_tail()
                    c2_q[0] = emit_c2(0)
                c2 = c2_q.pop(s)

                # packed h2: col-tiled matmuls, 4 rounds -> one PSUM bank
                hp = 32 * (R - 1) + H2
                ps2 = pp.tile([hp, 2 * MC], F32, tag="ah2")
                for t in range(NDT):
                    for g in range(R):
                        nc.tensor.matmul(ps2[32 * g:32 * g + H2, :],
                                         wsl(w1c, H, t, H1, H2),
                                         pts[t][:, poff + 2 * g * MC:
                                                poff + (2 * g + 2) * MC],
                                         start=(t == 0), stop=False,
                                         tile_position=(0, 32 * g))
                for g in range(R):
                    nc.tensor.matmul(ps2[32 * g:32 * g + H2, :],
                                     idn[0:H2, 0:H2],
                                     c2[0:H2, 2 * g * MC:(2 * g + 2) * MC],
                                     start=False, stop=True,
                                     tile_position=(0, 32 * g))
                a2x = wp.tile([hp, 2 * MC], F16, tag="a2x",
                              padded_shape=[128, 2 * MC])
                nc.scalar.activation(a2x[:], ps2[:], AF.Relu)

                # respread tail for SR s-1 (its DMA has landed by now)
                if prev is not None:
                    emit_respread_tail(prev, pairK_prev)

                # prefetch DVE work for s+2 / c2 for s+1
                if s + 1 < len(SRS):
                    c1_q[s + 1] = emit_c1(s + 1)
                    c2_q[s + 1] = emit_c2(s + 1)
                if s + 2 in PGROUPS:
                    emit_products(PGROUPS[s + 2])

                prev = (s, R, a1s, a2x[:])

            # ---- epilogue: pairs of the last superround + final masking ----
            # add-only mask: masked slots get score + (-1e9) which rounds to
            # exactly -1e9 in fp32 (|score| << ulp(1e9)); col K stays memset 0.
            # cols [0, k0l) are final once the loop's last respread lands, so
            # mask+store them early; only the last chunk rides the tail.
            pair_sb_last = emit_pairs_for(prev, respread_dma=False)
            sl, Rl = prev[0], prev[1]
            k0l = 2 * r0s[sl]
            nc.vector.tensor_add(
                _ap3(scm[:], 0, 128, 0, [(K + 1, 2), (1, k0l)]),
                _ap3(scm[:], 0, 128, 0, [(K + 1, 2), (1, k0l)]),
                _ap3(mm_ma[:], 0, 128, 0, [(K + 1, 2), (1, k0l)]))
            nc.sync.dma_start(
                out=AP(out_d.tensor, out_d.offset,
                       [[K + 1, 128], [128 * (K + 1), 2], [1, k0l]]),
                in_=_ap3(scm[:], 0, 128, 0, [(K + 1, 2), (1, k0l)]))
            # respread the last pairs via block transposes (no DMA round trip
            # on the tail): pair_sb rows {32r} -> scm cols k0l+2r+j
            nrow = 32 * (Rl - 1) + 1
            for mb in range(2):
                for j in range(2):
                    ptr = pp.tile([128, nrow], F16, tag="pre",
                                  padded_shape=[128, 128])
                    nc.tensor.transpose(
                        ptr[:],
                        pair_sb_last[0:nrow, j * MC + 128 * mb:
                                     j * MC + 128 * (mb + 1)],
                        idn[0:nrow, 0:nrow])
                    nc.scalar.activation(
                        _ap3(scm[:], 0, 128, (K + 1) * mb + k0l + j, [(2, Rl)]),
                        _ap3(ptr[:], 0, 128, 0, [(32, Rl)]),
                        AF.Identity, bias=ment_col[mb][:])
            ntail = K + 1 - k0l
            nc.vector.tensor_add(
                _ap3(scm[:], 0, 128, k0l, [(K + 1, 2), (1, ntail)]),
                _ap3(scm[:], 0, 128, k0l, [(K + 1, 2), (1, ntail)]),
                _ap3(mm_ma[:], 0, 128, k0l, [(K + 1, 2), (1, ntail)]))
            nc.sync.dma_start(
                out=AP(out_d.tensor, out_d.offset + k0l,
                       [[K + 1, 128], [128 * (K + 1), 2], [1, ntail]]),
                in_=_ap3(scm[:], 0, 128, k0l, [(K + 1, 2), (1, ntail)]))

    nc.compile()
    return nc


def _prep_inputs(inputs):
    X = np.ascontiguousarray(inputs["mention_reprs"], dtype=np.float32)
    assert X.shape == (M, D)
    w1p = np.asarray(inputs["w1p"], dtype=np.float32)
    W1a, W1b, W1c = w1p[:D], w1p[D:2 * D], w1p[2 * D:]
    w1m = np.asarray(inputs["w1m"], dtype=np.float32)

    def padD(w, cols):  # [D, cols] -> [DP, cols] fp16
        out = np.zeros((DP, cols), dtype=np.float16)
        out[:D] = w.astype(np.float16)
        return out

    xtp = np.zeros((DP, M + HB), dtype=np.float16)
    xtp[:D, HB:] = X.T.astype(np.float16)

    # preamble packs: wh1 = [w1a_h1 | w1b_h1 | w1m_h1]; wh2 = [w1a_h2 (0:22) |
    # pad (22:32) | w1m_h2 (32:54) | w1b_h2 (54:76)]
    wh1 = np.concatenate([W1a[:, :H1], W1b[:, :H1], w1m[:, :H1]], axis=1)
    wh2 = np.zeros((D, 76), dtype=np.float32)
    wh2[:, 0:H2] = W1a[:, H1:]
    wh2[:, 32:32 + H2] = w1m[:, H1:]
    wh2[:, 54:54 + H2] = W1b[:, H1:]

    b1m = np.asarray(inputs["b1m"], dtype=np.float32)
    b1p = np.asarray(inputs["b1p"], dtype=np.float32)
    bias_all = np.zeros((128, 4), dtype=np.float32)
    bias_all[:, 0] = b1m[:H1]
    bias_all[32:32 + H2, 1] = b1m[H1:]
    bias_all[:, 2] = b1p[:H1]
    bias_all[:H2, 3] = b1p[H1:]

    w2m = np.asarray(inputs["w2m"], dtype=np.float32)
    w2p = np.asarray(inputs["w2p"], dtype=np.float32)
    w2_all = np.zeros((128, 4), dtype=np.float16)
    w2_all[:, 0] = w2m[:H1].astype(np.float16)
    w2_all[32:32 + H2, 1] = w2m[H1:].astype(np.float16)
    w2_all[:, 2] = w2p[:H1].astype(np.float16)
    for g in range(4):
        w2_all[32 * g:32 * g + H2, 3] = w2p[H1:].astype(np.float16)
    sm16 = np.concatenate([w2_all, np.eye(128, dtype=np.float16)], axis=1)

    shared = {
        "wh1": padD(wh1, 384),
        "wh2": padD(wh2, 76),
        "w1c": padD(W1c, H),
        "bias_all": bias_all,
        "sm16": np.ascontiguousarray(sm16),
    }

    b2m = float(np.asarray(inputs["b2m"]).reshape(-1)[0])
    b2p = float(np.asarray(inputs["b2p"]).reshape(-1)[0])
    in_maps = []
    for c in range(NCORES):
        r0 = MC * c
        xt_c = np.ascontiguousarray(xtp[:, r0:r0 + W])
        madd = np.full((MC, K + 1), np.float32(b2p + 2.0 * b2m), dtype=np.float32)
        madd[:, K] = 0.0
        if c == 0:
            for i in range(min(K, MC)):
                madd[i, :K - i] = np.float32(-1e9)
        mm_ma = np.zeros((128, 2 * (K + 1)), dtype=np.float32)
        for mb in range(2):
            mm_ma[:, (K + 1) * mb:(K + 1) * (mb + 1)] = \
                madd[128 * mb:128 * (mb + 1)]
        in_maps.append({"xt": xt_c, "mm_ma": mm_ma, **shared})
    return in_maps


def _get_nc(inputs):
    if "nc" not in _cache:
        _cache["nc"] = _build()
    return _cache["nc"]


def _run(inputs, trace=False):
    assert int(np.asarray(inputs["K"])) == K
    nc = _get_nc(inputs)
    in_maps = _prep_inputs(inputs)
    res = run_bass_kernel_spmd(nc, in_maps, list(range(NCORES)), trace=trace)
    out = np.concatenate([res.results[c]["out"] for c in range(NCORES)], axis=0)
    return out.astype(np.float32), res


def kernel(**inputs) -> np.ndarray:
    out, _ = _run(inputs, trace=False)
    return out


# revision 35
# speedup vs baseline: 1.0067x; 1.0067x over previous
"""CorefScore kernel for 8 Trainium2 NeuronCores.

Shards the mention axis M=2048 across 8 cores (256 owned mentions plus a
64-row halo of preceding mentions). The banded pairwise MLP runs in
"superrounds" of up to 4 rounds (2 deltas each): the DVE emits one batched
shifted-product tensor_tensor per d-tile per superround ([128, 8*256] fp16,
2x perf mode); the PE contracts them with W1c in fp16 (fp32 PSUM); Ya+shifted
Yb sums are merged on the DVE (c1/c2) and injected via identity matmuls. The
22-wide h2 half of all rounds packs into one PSUM bank at partition offsets
0/32/64/96 via col-tiled matmuls on disjoint PE column strips. Pair scores
(w2p contraction) use diagonal 32-strips, packed per superround, respread via
DMA + PE transposes. v1: preamble packed into 5 weight streams (40 matmuls),
batched input DMAs with fine-grained xt chunks, first a1 round emitted ahead
of the deferred pair matmuls, product tile t=7 emitted on GpSimd to unload
the DVE, single fused output DMA.
"""

import os
import sys

import numpy as np

for _p in ("/opt/trn_rl_repo", "/opt/pypackages"):
    if os.path.isdir(_p) and _p not in sys.path:
        sys.path.append(_p)

import concourse.bacc as bacc
import concourse.bass as bass
import concourse.mybir as mybir
import concourse.tile as tile
from concourse.ap import AP
from concourse.bass_utils import run_bass_kernel_spmd

F16 = mybir.dt.float16
F32 = mybir.dt.float32
AF = mybir.ActivationFunctionType

M, D, H, K = 2048, 900, 150, 50
NCORES = 8
MC = M // NCORES          # owned mentions per core
HB = 64                   # halo columns (>= K)
W = MC + HB               # X^T window width per core
DP = 1024                 # padded feature dim (8 tiles of 128)
NDT = DP // 128           # number of d tiles
H1, H2 = 128, H - 128     # h split
# superround sizes (rounds of 2 deltas each); sum = 25 rounds = 50 deltas;
# last kept smallest so the epilogue pair/respread chain is short
SRS = [3, 4, 4, 4, 4, 4, 2]
GPS_T7 = False            # GpSimd tensor_tensor measured 2-8x slower + DVE
                          # SBUF-port contention; keep everything on the DVE

_cache = {}


def _ap3(t_ap, p_lo, p_n, off, dims, pstep=1):
    """3-D free-dim view of a tile AP: partitions [p_lo, p_lo+p_n*pstep) with
    partition step pstep, free offset `off` elements, free dims."""
    b = t_ap[p_lo:p_lo + 1, 0:1]
    pstride = b.ap[0][0]
    return AP(b.tensor, b.offset + off,
              [[pstride * pstep, p_n]] + [list(d) for d in dims])


def _build():
    nc = bacc.Bacc("TRN2", target_bir_lowering=False, debug=False)

    xt_d = nc.dram_tensor("xt", [DP, W], F16, kind="ExternalInput").ap()
    wh1_d = nc.dram_tensor("wh1", [DP, 384], F16, kind="ExternalInput").ap()
    wh2_d = nc.dram_tensor("wh2", [DP, 76], F16, kind="ExternalInput").ap()
    w1c_d = nc.dram_tensor("w1c", [DP, H], F16, kind="ExternalInput").ap()
    bias_d = nc.dram_tensor("bias_all", [128, 4], F32, kind="ExternalInput").ap()
    sm16_d = nc.dram_tensor("sm16", [128, 132], F16, kind="ExternalInput").ap()
    mm_d = nc.dram_tensor("mm_ma", [128, 2 * (K + 1)], F32,
                          kind="ExternalInput").ap()
    out_d = nc.dram_tensor("out", [MC, K + 1], F32, kind="ExternalOutput").ap()

    def dma3(eng, dst_tile, src_dram, cols, t0=0, t1=NDT):
        """One DMA loading d-tiles [t0, t1) of [DP, cols] dram into a
        [128, NDT*cols] tile at the matching free offset."""
        src = AP(src_dram.tensor, src_dram.offset + 128 * cols * t0,
                 [[cols, 128], [128 * cols, t1 - t0], [1, cols]])
        eng.dma_start(out=dst_tile[:, cols * t0:cols * t1], in_=src)

    with tile.TileContext(nc) as tc:
        with (
            tc.tile_pool(name="cp", bufs=1) as cp,
            tc.tile_pool(name="wp", bufs=2) as wp,
            tc.tile_pool(name="pp", bufs=2, space="PSUM") as pp,
        ):
            # ---- input loads ----
            # Two HWDGE queues, ordered so the two lead-in critical paths
            # (DVE product chain over xt, and wh1 -> psya/psyb -> c1) finish
            # together: sync carries xt tiles 0-5 + sm16 + w1c, scalar carries
            # wh1 + bias + xt tiles 6-7 + the rest.
            xt = cp.tile([128, NDT * W], F16, tag="xt")
            dma3(nc.sync, xt, xt_d, W, 0, 2)
            dma3(nc.sync, xt, xt_d, W, 2, 4)
            dma3(nc.sync, xt, xt_d, W, 4, 6)
            sm16 = cp.tile([128, 132], F16, tag="sm16")
            nc.sync.dma_start(out=sm16[:], in_=sm16_d[:])
            w1c = cp.tile([128, NDT * H], F16, tag="w1c")
            dma3(nc.sync, w1c, w1c_d, H, 0, 4)
            dma3(nc.sync, w1c, w1c_d, H, 4, NDT)
            bias = cp.tile([128, 4], F32, tag="bias")
            nc.scalar.dma_start(out=bias[:], in_=bias_d[:])
            wh1 = cp.tile([128, NDT * 384], F16, tag="wh1")
            dma3(nc.scalar, wh1, wh1_d, 384, 0, 2)
            dma3(nc.scalar, wh1, wh1_d, 384, 2, 5)
            dma3(nc.scalar, wh1, wh1_d, 384, 5, NDT)
            dma3(nc.scalar, xt, xt_d, W, 6, NDT)
            wh2 = cp.tile([128, NDT * 76], F16, tag="wh2")
            dma3(nc.scalar, wh2, wh2_d, 76)
            mm_ma = cp.tile([128, 2 * (K + 1)], F32, tag="mm_ma")
            nc.scalar.dma_start(out=mm_ma[:], in_=mm_d[:])

            w2 = sm16[:, 0:4]
            idn = sm16[:, 4:132]

            def wsl(wt, stride, t, ho, hn):  # stationary slice of a weight tile
                return wt[:, stride * t + ho:stride * t + ho + hn]

            # ---- preamble MLPs: 5 packed streams ----
            # wh1 = [w1a_h1 | w1b_h1 | w1m_h1], wh2 = [ya2 | pad | ma2 | yb2]
            psya1 = pp.tile([H1, MC], F32, tag="ah1")
            psyb1 = pp.tile([H1, W], F32, tag="ah1")
            psm1 = pp.tile([H1, W], F32, tag="ah2")
            psX = pp.tile([54, W], F32, tag="ah2")
            psyb2 = pp.tile([H2, W], F32, tag="psp")
            for t in range(NDT):
                nc.tensor.matmul(psya1[:], wsl(wh1, 384, t, 0, 128),
                                 xt[:, W * t + HB:W * (t + 1)],
                                 start=(t == 0), stop=(t == NDT - 1))
            for t in range(NDT):
                nc.tensor.matmul(psyb1[:], wsl(wh1, 384, t, 128, 128),
                                 xt[:, W * t:W * (t + 1)],
                                 start=(t == 0), stop=(t == NDT - 1))
            for t in range(NDT):
                nc.tensor.matmul(psm1[:], wsl(wh1, 384, t, 256, 128),
                                 xt[:, W * t:W * (t + 1)],
                                 start=(t == 0), stop=(t == NDT - 1))
            for t in range(NDT):
                nc.tensor.matmul(psX[:], wsl(wh2, 76, t, 0, 54),
                                 xt[:, W * t:W * (t + 1)],
                                 start=(t == 0), stop=(t == NDT - 1))
            for t in range(NDT):
                nc.tensor.matmul(psyb2[:], wsl(wh2, 76, t, 54, 22),
                                 xt[:, W * t:W * (t + 1)],
                                 start=(t == 0), stop=(t == NDT - 1))

            ya1 = cp.tile([H1, MC], F16, tag="ya1")
            nc.scalar.activation(ya1[:], psya1[:], AF.Identity, bias=bias[:, 2:3])
            yb1 = cp.tile([H1, W], F16, tag="yb1")
            nc.scalar.copy(yb1[:], psyb1[:])
            ya2 = cp.tile([H2, MC], F16, tag="ya2")
            nc.scalar.activation(ya2[:], psX[0:H2, HB:], AF.Identity,
                                 bias=bias[0:H2, 3:4])
            ma1 = cp.tile([H1, W], F16, tag="ma1")
            nc.scalar.activation(ma1[:], psm1[:], AF.Relu, bias=bias[:, 0:1])
            ma2t = cp.tile([54, W], F16, tag="ma2t")
            nc.scalar.activation(ma2t[32:54, :], psX[32:54, :], AF.Relu,
                                 bias=bias[32:54, 1:2])
            yb2 = cp.tile([H2, W], F16, tag="yb2")
            nc.scalar.copy(yb2[:], psyb2[:])

            # mention score row over the window
            psme = pp.tile([1, W], F32, tag="psp")
            nc.tensor.matmul(psme[:], w2[:, 0:1], ma1[:], start=True, stop=False)
            nc.tensor.matmul(psme[:], w2[32:54, 1:2], ma2t[32:54, :],
                             start=False, stop=True, tile_position=(32, 0))
            # row 0 = mention scores; rows 1..31 stay zero so the e0-column
            # identity-inject matmul (32-wide contraction) picks row 0 only
            ment_row = cp.tile([32, W], F16, tag="ment_row")
            nc.vector.memset(ment_row[:], 0.0)
            nc.scalar.copy(ment_row[0:1, :], psme[:])
            # ment as per-partition columns for the owned 2x128 mention blocks
            ment_col = []
            for mb in range(2):
                pst = pp.tile([128, 1], F16, tag="pre")
                nc.tensor.transpose(pst[:],
                                    ment_row[0:1, HB + 128 * mb:HB + 128 * (mb + 1)],
                                    idn[0:1, 0:1])
                mcol = cp.tile([128, 1], F32, tag=f"mcol{mb}")
                nc.scalar.copy(mcol[:], pst[:])
                ment_col.append(mcol)

            scm = cp.tile([128, 2 * (K + 1)], F32, tag="scm")
            nc.vector.memset(scm[:], 0.0)

            # ---- DVE: batched shifted products per superround ----
            # SR s covers rounds r0..r0+R-1; delta of column group j (0..2R-1)
            # is d0 - j with d0 = K - 2*r0; product col j*MC+m multiplies
            # X^T[., m] (owned) by X^T[., m - (d0 - j)].
            r0s = []
            acc = 0
            for R in SRS:
                r0s.append(acc)
                acc += R

            pts_q = {}

            def emit_products(ss):
                """One batched product op per d-tile covering the superrounds
                in `ss` (their delta ranges are contiguous)."""
                d0 = K - 2 * r0s[ss[0]]
                RT = sum(SRS[s] for s in ss)
                pts = []
                for t in range(NDT):
                    pt = wp.tile([128, 2 * RT * MC], F16, tag=f"pt{t}",
                                 padded_shape=[128, 8 * MC])
                    nc.vector.tensor_tensor(
                        _ap3(pt[:], 0, 128, 0, [(MC, 2 * RT), (1, MC)]),
                        _ap3(xt[:], 0, 128, W * t + HB, [(0, 2 * RT), (1, MC)]),
                        _ap3(xt[:], 0, 128, W * t + HB - d0,
                             [(1, 2 * RT), (1, MC)]),
                        mybir.AluOpType.mult)
                    pts.append(pt)
                off = 0
                for s in ss:
                    pts_q[s] = (pts, off)
                    off += 2 * SRS[s] * MC

            def emit_c2(s):
                R = SRS[s]
                d0 = K - 2 * r0s[s]
                c2 = wp.tile([H2, 2 * R * MC], F16, tag="c2",
                             padded_shape=[128, 8 * MC])
                nc.vector.tensor_tensor(
                    _ap3(c2[:], 0, H2, 0, [(MC, 2 * R), (1, MC)]),
                    _ap3(ya2[:], 0, H2, 0, [(0, 2 * R), (1, MC)]),
                    _ap3(yb2[:], 0, H2, HB - d0, [(1, 2 * R), (1, MC)]),
                    mybir.AluOpType.add)
                return c2

            def emit_c1(s):
                R = SRS[s]
                d0 = K - 2 * r0s[s]
                c1 = wp.tile([H1, 2 * R * MC], F16, tag="c1",
                             padded_shape=[128, 8 * MC])
                nc.vector.tensor_tensor(
                    _ap3(c1[:], 0, H1, 0, [(MC, 2 * R), (1, MC)]),
                    _ap3(ya1[:], 0, H1, 0, [(0, 2 * R), (1, MC)]),
                    _ap3(yb1[:], 0, H1, HB - d0, [(1, 2 * R), (1, MC)]),
                    mybir.AluOpType.add)
                return c1

            # per-SR product emission: merging SR pairs into bigger ops makes
            # the DVE FIFO lumpy and head-of-line-blocks the next SR's c1/c2
            PGROUPS = {s: (s,) for s in range(len(SRS))}
            emit_products(PGROUPS[0])
            c1_q = {0: emit_c1(0)}
            c2_q = {0: emit_c2(0)}
            emit_products(PGROUPS[1])

            # ---- superround loop state ----
            prev = None       # state of SR s-1 for deferred pair work

            def emit_pairs_for(state, respread_dma=True):
                """PE pair matmuls + evac + respread DMA for SR s-1."""
                s, R, a1s, a2xs = state
                d0 = K - 2 * r0s[s]
                psp = pp.tile([97, MC * 2], F32, tag="psp")
                # stage-ordered across the 4 column strips so the strips run
                # concurrently: all ment_j opens, then all w2-a1, then w2-a2x.
                for g in range(R):
                    # ment_j first (start=True): a strided-moving matmul with
                    # start=False wedges the exec unit, so it opens the group
                    nc.tensor.matmul(
                        psp[32 * g:32 * g + 1, :], idn[0:32, 0:1],
                        _ap3(ment_row[:], 0, 32, HB - (d0 - 2 * g),
                             [(1, 2), (1, MC)]),
                        start=True, stop=False, tile_position=(0, 32 * g))
                for g in range(R):
                    nc.tensor.matmul(psp[32 * g:32 * g + 1, :], w2[:, 2:3],
                                     a1s[g][:], start=False, stop=False,
                                     tile_position=(0, 32 * g))
                for g in range(R):
                    nc.tensor.matmul(psp[32 * g:32 * g + 1, :],
                                     w2[32 * g:32 * g + H2, 3:4],
                                     a2xs[32 * g:32 * g + H2, :],
                                     start=False, stop=True,
                                     tile_position=(32 * g, 32 * g))
                pair_sb = wp.tile([97, MC * 2], F16, tag="pair_sb")
                nc.scalar.copy(pair_sb[:], psp[:])
                if not respread_dma:
                    return pair_sb
                pairK = wp.tile([2 * R, MC], F16, tag="pairK",
                                padded_shape=[128, MC])
                nc.sync.dma_start(
                    out=pairK[:],
                    in_=_ap3(pair_sb[:], 0, R, 0, [(MC, 2), (1, MC)], pstep=32))
                return pairK

            def emit_respread_tail(state, pairK):
                """Transposes + scM chunk evac for SR s-1 (after its DMA)."""
                s, R, a1s, a2xs = state
                k0 = 2 * r0s[s]
                cn = 2 * R
                for mb in range(2):
                    ptr = pp.tile([128, cn], F16, tag="pre",
                                  padded_shape=[128, 8])
                    nc.tensor.transpose(ptr[:], pairK[0:cn, 128 * mb:128 * (mb + 1)],
                                        idn[0:cn, 0:cn])
                    nc.scalar.activation(
                        scm[:, (K + 1) * mb + k0:(K + 1) * mb + k0 + cn],
                        ptr[:], AF.Identity, bias=ment_col[mb][:])

            def emit_a1_round(g, pts, poff, c1):
                ps1 = pp.tile([H1, 2 * MC], F32, tag="ah1")
                for t in range(NDT):
                    nc.tensor.matmul(ps1[:], wsl(w1c, H, t, 0, H1),
                                     pts[t][:, poff + 2 * g * MC:
                                            poff + (2 * g + 2) * MC],
                                     start=(t == 0), stop=False)
                nc.tensor.matmul(
                    ps1[:], idn[0:128, 0:128],
                    c1[:, 2 * g * MC:(2 * g + 2) * MC],
                    start=False, stop=True)
                a1 = wp.tile([H1, 2 * MC], F16, tag="a1", bufs=8)
                nc.scalar.activation(a1[:], ps1[:], AF.Relu)
                return a1

            for s, R in enumerate(SRS):
                pts, poff = pts_q.pop(s)
                c1 = c1_q.pop(s)

                # first round ahead of the deferred pair matmuls so the PE has
                # work while the previous SR's a2x evac lands
                a1s = [emit_a1_round(0, pts, poff, c1)]
                pairK_prev = emit_pairs_for(prev) if prev is not None else None
                for g in range(1, R):
                    a1s.append(emit_a1_round(g, pts, poff, c1))
                c2 = c2_q.pop(s)

                # packed h2: col-tiled matmuls, 4 rounds -> one PSUM bank
                hp = 32 * (R - 1) + H2
                ps2 = pp.tile([hp, 2 * MC], F32, tag="ah2")
                for t in range(NDT):
                    for g in range(R):
                        nc.tensor.matmul(ps2[32 * g:32 * g + H2, :],
                                         wsl(w1c, H, t, H1, H2),
                                         pts[t][:, poff + 2 * g * MC:
                                                poff + (2 * g + 2) * MC],
                                         start=(t == 0), stop=False,
                                         tile_position=(0, 32 * g))
                for g in range(R):
                    nc.tensor.matmul(ps2[32 * g:32 * g + H2, :],
                                     idn[0:H2, 0:H2],
                                     c2[0:H2, 2 * g * MC:(2 * g + 2) * MC],
                                     start=False, stop=True,
                                     tile_position=(0, 32 * g))
                a2x = wp.tile([hp, 2 * MC], F16, tag="a2x",
                              padded_shape=[128, 2 * MC])
                nc.scalar.activation(a2x[:], ps2[:], AF.Relu)

                # respread tail for SR s-1 (its DMA has landed by now)
                if prev is not None:
                    emit_respread_tail(prev, pairK_prev)

                # prefetch DVE work for s+2 / c2 for s+1
                if s + 1 < len(SRS):
                    c1_q[s + 1] = emit_c1(s + 1)
                    c2_q[s + 1] = emit_c2(s + 1)
                if s + 2 in PGROUPS:
                    emit_products(PGROUPS[s + 2])

                prev = (s, R, a1s, a2x[:])

            # ---- epilogue: pairs of the last superround + final masking ----
            # add-only mask: masked slots get score + (-1e9) which rounds to
            # exactly -1e9 in fp32 (|score| << ulp(1e9)); col K stays memset 0.
            # cols [0, k0l) are final once the loop's last respread lands, so
            # mask+store them early; only the last chunk rides the tail.
            pair_sb_last = emit_pairs_for(prev, respread_dma=False)
            sl, Rl = prev[0], prev[1]
            k0l = 2 * r0s[sl]
            nc.vector.tensor_add(
                _ap3(scm[:], 0, 128, 0, [(K + 1, 2), (1, k0l)]),
                _ap3(scm[:], 0, 128, 0, [(K + 1, 2), (1, k0l)]),
                _ap3(mm_ma[:], 0, 128, 0, [(K + 1, 2), (1, k0l)]))
            nc.sync.dma_start(
                out=AP(out_d.tensor, out_d.offset,
                       [[K + 1, 128], [128 * (K + 1), 2], [1, k0l]]),
                in_=_ap3(scm[:], 0, 128, 0, [(K + 1, 2), (1, k0l)]))
            # respread the last pairs via block transposes (no DMA round trip
            # on the tail): pair_sb rows {32r} -> scm cols k0l+2r+j
            nrow = 32 * (Rl - 1) + 1
            for mb in range(2):
                for j in range(2):
                    ptr = pp.tile([128, nrow], F16, tag="pre",
                                  padded_shape=[128, 128])
                    nc.tensor.transpose(
                        ptr[:],
                        pair_sb_last[0:nrow, j * MC + 128 * mb:
                                     j * MC + 128 * (mb + 1)],
                        idn[0:nrow, 0:nrow])
                    nc.scalar.activation(
                        _ap3(scm[:], 0, 128, (K + 1) * mb + k0l + j, [(2, Rl)]),
                        _ap3(ptr[:], 0, 128, 0, [(32, Rl)]),
                        AF.Identity, bias=ment_col[mb][:])
            ntail = K + 1 - k0l
            nc.vector.tensor_add(
                _ap3(scm[:], 0, 128, k0l, [(K + 1, 2), (1, ntail)]),
                _ap3(scm[:], 0, 128, k0l, [(K + 1, 2), (1, ntail)]),
                _ap3(mm_ma[:], 0, 128, k0l, [(K + 1, 2), (1, ntail)]))
            nc.sync.dma_start(
                out=AP(out_d.tensor, out_d.offset + k0l,
                       [[K + 1, 128], [128 * (K + 1), 2], [1, ntail]]),
                in_=_ap3(scm[:], 0, 128, k0l, [(K + 1, 2), (1, ntail)]))

    nc.compile()
    return nc


def _prep_inputs(inputs):
    X = np.ascontiguousarray(inputs["mention_reprs"], dtype=np.float32)
    assert X.shape == (M, D)
    w1p = np.asarray(inputs["w1p"], dtype=np.float32)
    W1a, W1b, W1c = w1p[:D], w1p[D:2 * D], w1p[2 * D:]
    w1m = np.asarray(inputs["w1m"], dtype=np.float32)

    def padD(w, cols):  # [D, cols] -> [DP, cols] fp16
        out = np.zeros((DP, cols), dtype=np.float16)
        out[:D] = w.astype(np.float16)
        return out

    xtp = np.zeros((DP, M + HB), dtype=np.float16)
    xtp[:D, HB:] = X.T.astype(np.float16)

    # preamble packs: wh1 = [w1a_h1 | w1b_h1 | w1m_h1]; wh2 = [w1a_h2 (0:22) |
    # pad (22:32) | w1m_h2 (32:54) | w1b_h2 (54:76)]
    wh1 = np.concatenate([W1a[:, :H1], W1b[:, :H1], w1m[:, :H1]], axis=1)
    wh2 = np.zeros((D, 76), dtype=np.float32)
    wh2[:, 0:H2] = W1a[:, H1:]
    wh2[:, 32:32 + H2] = w1m[:, H1:]
    wh2[:, 54:54 + H2] = W1b[:, H1:]

    b1m = np.asarray(inputs["b1m"], dtype=np.float32)
    b1p = np.asarray(inputs["b1p"], dtype=np.float32)
    bias_all = np.zeros((128, 4), dtype=np.float32)
    bias_all[:, 0] = b1m[:H1]
    bias_all[32:32 + H2, 1] = b1m[H1:]
    bias_all[:, 2] = b1p[:H1]
    bias_all[:H2, 3] = b1p[H1:]

    w2m = np.asarray(inputs["w2m"], dtype=np.float32)
    w2p = np.asarray(inputs["w2p"], dtype=np.float32)
    w2_all = np.zeros((128, 4), dtype=np.float16)
    w2_all[:, 0] = w2m[:H1].astype(np.float16)
    w2_all[32:32 + H2, 1] = w2m[H1:].astype(np.float16)
    w2_all[:, 2] = w2p[:H1].astype(np.float16)
    for g in range(4):
        w2_all[32 * g:32 * g + H2, 3] = w2p[H1:].astype(np.float16)
    sm16 = np.concatenate([w2_all, np.eye(128, dtype=np.float16)], axis=1)

    shared = {
        "wh1": padD(wh1, 384),
        "wh2": padD(wh2, 76),
        "w1c": padD(W1c, H),
        "bias_all": bias_all,
        "sm16": np.ascontiguousarray(sm16),
    }

    b2m = float(np.asarray(inputs["b2m"]).reshape(-1)[0])
    b2p = float(np.asarray(inputs["b2p"]).reshape(-1)[0])
    in_maps = []
    for c in range(NCORES):
        r0 = MC * c
        xt_c = np.ascontiguousarray(xtp[:, r0:r0 + W])
        madd = np.full((MC, K + 1), np.float32(b2p + 2.0 * b2m), dtype=np.float32)
        madd[:, K] = 0.0
        if c == 0:
            for i in range(min(K, MC)):
                madd[i, :K - i] = np.float32(-1e9)
        mm_ma = np.zeros((128, 2 * (K + 1)), dtype=np.float32)
        for mb in range(2):
            mm_ma[:, (K + 1) * mb:(K + 1) * (mb + 1)] = \
                madd[128 * mb:128 * (mb + 1)]
        in_maps.append({"xt": xt_c, "mm_ma": mm_ma, **shared})
    return in_maps


def _get_nc(inputs):
    if "nc" not in _cache:
        _cache["nc"] = _build()
    return _cache["nc"]


def _run(inputs, trace=False):
    assert int(np.asarray(inputs["K"])) == K
    nc = _get_nc(inputs)
    in_maps = _prep_inputs(inputs)
    res = run_bass_kernel_spmd(nc, in_maps, list(range(NCORES)), trace=trace)
    out = np.concatenate([res.results[c]["out"] for c in range(NCORES)], axis=0)
    return out.astype(np.float32), res


def kernel(**inputs) -> np.ndarray:
    out, _ = _run(inputs, trace=False)
    return out
